# revision 1
# baseline (speedup 1.0000x reference)
"""DGCNN (4-layer EdgeConv + head) Bass kernel for 8 Trainium2 NeuronCores.

Problem: nn_DGCNN_net (B=4, N=2048, K=32), eval-mode BN.

Sharding: 2 cores per batch element (data parallel over B=4 x 2-way split of
the N=2048 points). Cores 2b,2b+1 process batch b; even core owns points
0..1023, odd core 1024..2047. After each of the first three EdgeConv layers
the pair exchanges its half of the new features with a pair-wise AllGather;
the global max-pool is combined with a pair-wise AllReduce(max).

Exact algebraic restructuring of EdgeConv (valid since the folded BN scale is
positive and LeakyReLU/max are monotone):
    edge_conv(x)[o,n] = leaky( max_k y[o, idx[n,k]] + z[o,n] )
with  y = (s*Wnbr) @ x,  z = (s*(Wctr-Wnbr)) @ x + b
so the K-neighbor gather moves O(N*K*O) bytes but only O(N*C*O) flops.

Per layer on each core:
  1. pd[i,j] = 2 x_i.x_j - |x_i|^2 - |x_j|^2 for its 1024 rows (single fused
     PE matmul with augmented operands [x; xx; 1] x [2x; -1; -xx]).
  2. exact top-32 per row on the Vector engine (MAX8 / FIND_INDEX8 /
     MATCH_REPLACE rounds) -> [128, 32] uint32 index tiles.
  3. indirect DMA gather of y rows (y stored [N, O] in DRAM), the index tile
     drives descriptor generation; then max over the 32 gathered rows and the
     +z / LeakyReLU epilogue.
"""

import numpy as np

EPS = 1e-5
K = 32
N = 2048
B = 4
NCORES = 8
HALF = N // 2

# layer configs: (C_in, O_out)
LAYERS = [(3, 64), (64, 64), (64, 128), (128, 256)]

NEG_BIG = -3.0e38


def _build_program(n_cores: int, debug: bool = False):
    import concourse.bass as bass
    import concourse.mybir as mybir
    import concourse.bacc as bacc
    import concourse.tile as tile
    from concourse.bass import IndirectOffsetOnAxis, ds, ts

    fp32 = mybir.dt.float32
    u32 = mybir.dt.uint32
    AF = mybir.ActivationFunctionType
    OP = mybir.AluOpType

    nc = bacc.Bacc(
        "TRN2",
        target_bir_lowering=False,
        debug=False,
        num_devices=n_cores,
    )

    # ---------------- external IO ----------------
    def din(name, shape):
        return nc.dram_tensor(name, shape, fp32, kind="ExternalInput")

    x0 = din("x0", [3, N])
    wy_d = [din(f"wy{li}", [c, o]) for li, (c, o) in enumerate(LAYERS)]
    wz_d = [din(f"wz{li}", [c + 2, o]) for li, (c, o) in enumerate(LAYERS)]
    w5 = din("w5", [512, 1024])
    b5r = din("b5r", [1, 1024])
    w6T = din("w6T", [5, 64])
    b6c = din("b6c", [64, 1])
    w7T = din("w7T", [7, 64])
    b7c = din("b7c", [64, 1])
    L1T = din("L1T", [1152, 512])
    b8r = din("b8r", [1, 512])
    L2T = din("L2T", [512, 256])
    b9r = din("b9r", [1, 256])
    L3T = din("L3T", [256, 28])
    bL3r = din("bL3r", [1, 28])
    lvec = din("lvec", [5, 1])
    nvec = din("nvec", [7, 1])
    ident = din("ident", [128, 128])

    out_t = nc.dram_tensor("out", [1, 28], fp32, kind="ExternalOutput")
    dbg = {}
    if debug:
        dbg["pd"] = nc.dram_tensor("dbg_pd", [128, N], fp32, kind="ExternalOutput")
        dbg["idx"] = nc.dram_tensor("dbg_idx", [128, K], u32, kind="ExternalOutput")
        dbg["gm"] = nc.dram_tensor("dbg_gm", [128, K * 64], fp32, kind="ExternalOutput")
        dbg["xo"] = nc.dram_tensor("dbg_xo", [128, 64], fp32, kind="ExternalOutput")
        dbg["y"] = nc.dram_tensor("dbg_y", [N, 64], fp32, kind="ExternalOutput")
        dbg["amy"] = nc.dram_tensor("dbg_amy", [3, HALF], fp32, kind="ExternalOutput")
        dbg["x1"] = nc.dram_tensor("dbg_x1", [64, N], fp32, kind="ExternalOutput")

    groups = [[2 * i, 2 * i + 1] for i in range(max(1, n_cores // 2))]

    with tile.TileContext(nc) as tc:
        pid = nc.partition_id()
        off = (pid & 1) * HALF  # this core's first point row

        # ---------------- pools ----------------
        consts = tc.alloc_tile_pool(name="consts", bufs=1)
        xcmp = tc.alloc_tile_pool(name="xcmp", bufs=1)
        dramp = tc.alloc_tile_pool(name="dramp", bufs=1, space="DRAM")
        # layer-phase pools (released before the head phase)
        w1 = tc.alloc_tile_pool(name="w1", bufs=1)
        w2 = tc.alloc_tile_pool(name="w2", bufs=2)
        w3 = tc.alloc_tile_pool(name="w3", bufs=3)
        psA = tc.alloc_tile_pool(name="psA", bufs=1, space="PSUM")
        psC = tc.alloc_tile_pool(name="psC", bufs=1, space="PSUM")

        # ---------------- persistent tiles ----------------
        ident_sb = consts.tile([128, 128], fp32, name="ident_sb")
        nc.sync.dma_start(ident_sb[:], ident[:, :])
        ones_col = consts.tile([128, 1], fp32, name="ones_col")
        nc.vector.memset(ones_col[:], 1.0)

        wy_sb, wz_sb = [], []
        for li, (c, o) in enumerate(LAYERS):
            t1 = consts.tile([c, o], fp32, name=f"wy_sb{li}")
            nc.sync.dma_start(t1[:], wy_d[li][:, :])
            wy_sb.append(t1)
            if c + 2 <= 128:
                t2 = consts.tile([c + 2, o], fp32, name=f"wz_sb{li}")
                nc.sync.dma_start(t2[:], wz_d[li][:, :])
                wz_sb.append([(0, c + 2, t2)])
            else:
                t2a = consts.tile([128, o], fp32, name=f"wz_sba{li}")
                nc.sync.dma_start(t2a[:], wz_d[li][0:128, :])
                t2b = consts.tile([c + 2 - 128, o], fp32, name=f"wz_sbb{li}")
                nc.sync.dma_start(t2b[:], wz_d[li][128:c + 2, :])
                wz_sb.append([(0, 128, t2a), (128, c + 2, t2b)])

        # channel-major layer inputs (kept alive: layers 2..4 inputs double as
        # x1,x2,x3 for the head concat)
        x_cm = [
            xcmp.tile([c, N], fp32, name=f"x_cm{li}")
            for li, (c, o) in enumerate(LAYERS)
        ]
        nc.sync.dma_start(x_cm[0][:], x0[:, :])
        # my half of layer-4 output, channel-major, as two [128, 1024] tiles
        x4_my = [xcmp.tile([128, HALF], fp32, name=f"x4_my{j}") for j in range(2)]

        # ---------------- DRAM scratch ----------------
        y_dram = [
            dramp.tile([N, o], fp32, name=f"y_dram{li}")
            for li, (c, o) in enumerate(LAYERS)
        ]
        xchg_in = [
            dramp.tile([HALF, o], fp32, name=f"xchg_in{li}")
            for li, (c, o) in enumerate(LAYERS[:3])
        ]
        xchg_out = [
            dramp.tile([N, o], fp32, name=f"xchg_out{li}")
            for li, (c, o) in enumerate(LAYERS[:3])
        ]
        hred_in = dramp.tile([128, 8], fp32, name="hred_in")
        hred_out = dramp.tile([128, 8], fp32, name="hred_out")

        # ================= EdgeConv layers =================
        def edge_layer(li, c, o):
            last = li == 3
            xc = x_cm[li]

            # ---- augmented operands ----
            # A = [x ; xx ; 1] (my half),  Bt = [2x ; -1 ; -xx] (all points)
            xx_row = w1.tile([1, N], fp32, tag="xx_row")
            ones_row = w1.tile([1, N], fp32, tag="ones_row")
            nc.vector.memset(ones_row[:], 1.0)
            if c + 2 <= 128:
                bt = w1.tile([c + 2, N], fp32, tag="bt")
                bt_slices = [(bt, 0, c + 2)]
            else:
                bt = w1.tile([c, N], fp32, tag="bt")
                bt_aux = w1.tile([2, N], fp32, tag="bt_aux")
                bt_slices = [(bt, 0, c), (bt_aux, c, c + 2)]
            for q in range(4):
                xsq = w2.tile([c, 512], fp32, tag="xsq")
                nc.scalar.activation(xsq[:], xc[:, ts(q, 512)], AF.Square)
                xx_ps = psC.tile([1, 512], fp32, tag="xx_ps")
                nc.tensor.matmul(
                    xx_ps[:], ones_col[:c, :], xsq[:], start=True, stop=True
                )
                nc.scalar.copy(xx_row[:, ts(q, 512)], xx_ps[:])
                nc.scalar.activation(
                    bt[0:c, ts(q, 512)], xc[:, ts(q, 512)], AF.Copy, scale=2.0
                )
            neg1_row = w1.tile([1, N], fp32, tag="neg1_row")
            nc.vector.memset(neg1_row[:], -1.0)
            negxx_row = w1.tile([1, N], fp32, tag="negxx_row")
            nc.scalar.activation(negxx_row[:], xx_row[:], AF.Copy, scale=-1.0)
            bt_last = bt_slices[-1][0]
            aux_base = c if len(bt_slices) == 1 else 0
            nc.sync.dma_start(bt_last[aux_base:aux_base + 1, :], neg1_row[:])
            nc.sync.dma_start(bt_last[aux_base + 1:aux_base + 2, :], negxx_row[:])

            if c + 2 <= 128:
                a_my = w1.tile([c + 2, HALF], fp32, tag="a_my")
                nc.sync.dma_start(a_my[0:c, :], xc[:, ds(off, HALF)])
                nc.sync.dma_start(a_my[c:c + 1, :], xx_row[:, ds(off, HALF)])
                nc.sync.dma_start(a_my[c + 1:c + 2, :], ones_row[0:1, 0:HALF])
                amy_parts = [(0, c, a_my), (0, c + 2, a_my)]  # [x-only, full]
                a_slices = [(a_my, 0, c + 2)]
            else:
                a_my = w1.tile([c, HALF], fp32, tag="a_my")
                nc.sync.dma_start(a_my[0:c, :], xc[:, ds(off, HALF)])
                a_aux = w1.tile([2, HALF], fp32, tag="a_aux")
                nc.sync.dma_start(a_aux[0:1, :], xx_row[:, ds(off, HALF)])
                nc.sync.dma_start(a_aux[1:2, :], ones_row[0:1, 0:HALF])
                a_slices = [(a_my, 0, c), (a_aux, c, c + 2)]
            if debug and li == 0:
                nc.sync.dma_start(dbg["amy"][:, :], a_my[0:3, :])

            # ---- y projections for ALL N points -> DRAM ----
            for j in range(16):
                y_ps = psC.tile([128, o], fp32, tag="y_ps")
                nc.tensor.matmul(
                    y_ps[:], xc[:, ts(j, 128)], wy_sb[li][:], start=True, stop=True
                )
                y_sb = w3.tile([128, o], fp32, tag="y_sb")
                nc.scalar.copy(y_sb[:], y_ps[:])
                nc.sync.dma_start(y_dram[li][ts(j, 128), :], y_sb[:])
                if debug and li == 0:
                    nc.sync.dma_start(dbg["y"][ts(j, 128), :], y_sb[:])

            # ---- per point-block of my half ----
            for i in range(8):
                # z for my block rows
                z_ps = psC.tile([128, o], fp32, tag="z_ps")
                nparts = wz_sb[li]
                for pi, ((a_t, a0, a1), (r0, r1, wt)) in enumerate(
                    zip(a_slices, nparts)
                ):
                    nc.tensor.matmul(
                        z_ps[:], a_t[:, ts(i, 128)], wt[:],
                        start=(pi == 0), stop=(pi == len(nparts) - 1),
                    )
                z_sb = w2.tile([128, o], fp32, tag="z_sb")
                nc.scalar.copy(z_sb[:], z_ps[:])
                # y rows of my own block (the k=0 self neighbor, no gather)
                ymy_ps = psC.tile([128, o], fp32, tag="z_ps")
                nc.tensor.matmul(
                    ymy_ps[:], a_slices[0][0][0:c, ts(i, 128)], wy_sb[li][:],
                    start=True, stop=True,
                )
                ymy = w2.tile([128, o], fp32, tag="ymy")
                nc.scalar.copy(ymy[:], ymy_ps[:])

                # pd = 2 x_i.x_j - xx_i - xx_j (PSUM, fp32)
                pd_ps = psA.tile([128, N], fp32, tag="pd_ps")
                for q in range(4):
                    for pi, ((a_t, a0, a1), (b_t, b0, b1)) in enumerate(
                        zip(a_slices, bt_slices)
                    ):
                        nc.tensor.matmul(
                            pd_ps[:, ts(q, 512)],
                            a_t[:, ts(i, 128)] if a_t.shape[0] == a1 - a0
                            else a_t[a0:a1, ts(i, 128)],
                            b_t[:, ts(q, 512)],
                            start=(pi == 0), stop=(pi == len(a_slices) - 1),
                        )
                pd_sb = w2.tile([128, N], fp32, tag="pd_sb", bufs=3)
                nc.scalar.copy(pd_sb[:], pd_ps[:])
                if debug and li == 0 and i == 0:
                    nc.sync.dma_start(dbg["pd"][:, :], pd_sb[:])

                # ---- exact top-32 per row (DVE) ----
                idx_t = w2.tile([128, K], u32, tag="idx_t")
                for r in range(4):
                    v8 = w2.tile([128, 8], fp32, tag="v8", bufs=4)
                    nc.vector.max(v8[:], pd_sb[:])
                    nc.vector.max_index(
                        idx_t[:, r * 8:(r + 1) * 8], v8[:], pd_sb[:]
                    )
                    if r < 3:
                        nc.vector.match_replace(pd_sb[:], v8[:], pd_sb[:], NEG_BIG)

                # ---- gather 32 y-rows per point, reduce, epilogue ----
                if debug and li == 0 and i == 0:
                    nc.sync.dma_start(dbg["idx"][:, :], idx_t[:])
                mx = w2.tile([128, o], fp32, tag="mx", bufs=3)
                nc.vector.tensor_copy(mx[:], ymy[:])
                for k in range(1, K):
                    offcol = w3.tile([128, 1], u32, tag="offcol", bufs=12)
                    nc.vector.tensor_copy(offcol[:], idx_t[:, k:k + 1])
                    gk = w3.tile([128, o], fp32, tag="gk", bufs=12)
                    nc.gpsimd.indirect_dma_start(
                        gk[:, :], None, y_dram[li][:, :],
                        IndirectOffsetOnAxis(ap=offcol[:, :], axis=0),
                    )
                    nc.vector.tensor_tensor(mx[:], mx[:], gk[:], op=OP.max)
                xo = w2.tile([128, o], fp32, tag="xo")
                nc.vector.tensor_tensor(xo[:], mx[:], z_sb[:], op=OP.add)
                nc.vector.scalar_tensor_tensor(
                    xo[:], xo[:], 0.2, xo[:], op0=OP.mult, op1=OP.max
                )

                if debug and li == 0 and i == 0:
                    nc.sync.dma_start(dbg["xo"][:, :], xo[:])
                if not last:
                    nc.sync.dma_start(xchg_in[li][ts(i, 128), :], xo[:])
                else:
                    for jj in range(2):
                        tp = psC.tile([128, 128], fp32, tag="tp")
                        nc.tensor.transpose(tp[:], xo[:, ts(jj, 128)], ident_sb[:])
                        nc.scalar.copy(x4_my[jj][:, ts(i, 128)], tp[:])

            if not last:
                # ---- pair AllGather; rebuild channel-major x ----
                if n_cores == 1:
                    nc.sync.dma_start(xchg_out[li][0:HALF, :], xchg_in[li][:, :])
                    nc.sync.dma_start(xchg_out[li][HALF:N, :], xchg_in[li][:, :])
                else:
                    nc.gpsimd.collective_compute(
                        "AllGather",
                        OP.bypass,
                        replica_groups=groups,
                        ins=[xchg_in[li][:, :]],
                        outs=[xchg_out[li][:, :]],
                    )
                nxt = li + 1
                nblk = max(1, o // 128)
                for j in range(16):
                    xt_sb = w3.tile([128, o], fp32, tag="xt_sb")
                    nc.sync.dma_start(xt_sb[:], xchg_out[li][ts(j, 128), :])
                    for jj in range(nblk):
                        w = min(128, o)
                        tp = psC.tile([128, 128], fp32, tag="tp")
                        nc.tensor.transpose(
                            tp[:w, :], xt_sb[:, jj * 128:jj * 128 + w], ident_sb[:]
                        )
                        nc.scalar.copy(
                            x_cm[nxt][jj * 128:jj * 128 + w, ts(j, 128)], tp[:w, :]
                        )

        for li, (c, o) in enumerate(LAYERS):
            edge_layer(li, c, o)
            if debug and li == 0:
                nc.sync.dma_start(dbg["x1"][:, :], x_cm[1][:])

        # ================= head =================
        w3.release()
        w2.release()
        w1.release()
        psC.release()
        psA.release()
        w1 = tc.alloc_tile_pool(name="hw1", bufs=1)
        psA = tc.alloc_tile_pool(name="hpsA", bufs=1, space="PSUM")
        psC = tc.alloc_tile_pool(name="hpsC", bufs=1, space="PSUM")
        x1_my = w1.tile([64, HALF], fp32, tag="x1_my")
        nc.sync.dma_start(x1_my[:], x_cm[1][:, ds(off, HALF)])
        x2_my = w1.tile([64, HALF], fp32, tag="x2_my")
        nc.sync.dma_start(x2_my[:], x_cm[2][:, ds(off, HALF)])
        x3_my = w1.tile([128, HALF], fp32, tag="x3_my")
        nc.sync.dma_start(x3_my[:], x_cm[3][:, ds(off, HALF)])

        w5_sb = []
        for k, (r0, r1) in enumerate([(0, 64), (64, 128), (128, 256),
                                      (256, 384), (384, 512)]):
            t = w1.tile([r1 - r0, 1024], fp32, tag=f"w5_{k}")
            nc.sync.dma_start(t[:], w5[r0:r1, :])
            w5_sb.append(t)

        hmax = w1.tile([128, 1024], fp32, tag="hmax")
        for i in range(8):
            h_ps = psA.tile([128, 1024], fp32, tag="h_ps")
            lhs = [x1_my[:, ts(i, 128)], x2_my[:, ts(i, 128)],
                   x3_my[:, ts(i, 128)], x4_my[0][:, ts(i, 128)],
                   x4_my[1][:, ts(i, 128)]]
            for q in range(2):
                for ci, l_ap in enumerate(lhs):
                    nc.tensor.matmul(
                        h_ps[:, ts(q, 512)], l_ap,
                        w5_sb[ci][:, q * 512:(q + 1) * 512],
                        start=(ci == 0), stop=(ci == len(lhs) - 1),
                    )
            if i == 0:
                nc.scalar.copy(hmax[:], h_ps[:])
            else:
                nc.vector.tensor_tensor(hmax[:], h_ps[:], hmax[:], op=OP.max)

        # partition-reduce via transposes -> [128, 8] (chan 128*j+p at [p, j])
        hcat = w1.tile([128, 8], fp32, tag="hcat")
        for j in range(8):
            tp = psC.tile([128, 128], fp32, tag="tp")
            nc.tensor.transpose(tp[:], hmax[:, ts(j, 128)], ident_sb[:])
            nc.vector.tensor_reduce(
                hcat[:, j:j + 1], tp[:], axis=mybir.AxisListType.X, op=OP.max
            )
        nc.sync.dma_start(hred_in[:, :], hcat[:])
        if n_cores == 1:
            nc.sync.dma_start(hred_out[:, :], hred_in[:, :])
        else:
            nc.gpsimd.collective_compute(
                "AllReduce", OP.max, replica_groups=groups,
                ins=[hred_in[:, :]], outs=[hred_out[:, :]],
            )
        hfull = w1.tile([128, 8], fp32, tag="hfull")
        nc.sync.dma_start(hfull[:], hred_out[:, :])
        b5_sb = consts.tile([128, 8], fp32, name="b5_sb")
        nc.sync.dma_start(
            b5_sb[:], b5r.ap().rearrange("o (j p) -> (o p) j", p=128)
        )
        nc.vector.tensor_tensor(hfull[:], hfull[:], b5_sb[:], op=OP.add)
        nc.vector.scalar_tensor_tensor(
            hfull[:], hfull[:], 0.2, hfull[:], op0=OP.mult, op1=OP.max
        )

        # lf / nf columns
        lvec_sb = consts.tile([5, 1], fp32, name="lvec_sb")
        nc.sync.dma_start(lvec_sb[:], lvec[:, :])
        nvec_sb = consts.tile([7, 1], fp32, name="nvec_sb")
        nc.sync.dma_start(nvec_sb[:], nvec[:, :])
        w6_sb = consts.tile([5, 64], fp32, name="w6_sb")
        nc.sync.dma_start(w6_sb[:], w6T[:, :])
        w7_sb = consts.tile([7, 64], fp32, name="w7_sb")
        nc.sync.dma_start(w7_sb[:], w7T[:, :])
        b6_sb = consts.tile([64, 1], fp32, name="b6_sb")
        nc.sync.dma_start(b6_sb[:], b6c[:, :])
        b7_sb = consts.tile([64, 1], fp32, name="b7_sb")
        nc.sync.dma_start(b7_sb[:], b7c[:, :])

        def matvec_col(w_sb, v_sb, b_sb, n_out, tag):
            ps = psC.tile([n_out, 1], fp32, tag="tpv")
            nc.tensor.matmul(ps[:], w_sb[:], v_sb[:], start=True, stop=True)
            r = w1.tile([n_out, 1], fp32, tag=tag)
            nc.vector.tensor_tensor(r[:], ps[:], b_sb[:], op=OP.add)
            nc.vector.scalar_tensor_tensor(
                r[:], r[:], 0.2, r[:], op0=OP.mult, op1=OP.max
            )
            return r

        lf_sb = matvec_col(w6_sb, lvec_sb, b6_sb, 64, "lf_sb")
        nf_sb = matvec_col(w7_sb, nvec_sb, b7_sb, 64, "nf_sb")

        # u tile [128, 9]: cols 0..7 = h, col 8 = [lf ; nf]
        u_t = w1.tile([128, 9], fp32, tag="u_t")
        nc.vector.tensor_copy(u_t[:, 0:8], hfull[:])
        nc.sync.dma_start(u_t[0:64, 8:9], lf_sb[:])
        nc.sync.dma_start(u_t[64:128, 8:9], nf_sb[:])

        def fc_row(v_cols, n_ch, wT_d, n_out, b_d, relu, tag):
            """out [1, n_out] = v.T @ wT ; v given as [128, n_ch] columns."""
            w_sb = w1.tile([128, n_ch, n_out], fp32, tag=f"{tag}_w")
            nc.sync.dma_start(
                w_sb[:], wT_d.ap().rearrange("(ch p) f -> p ch f", p=128)
            )
            ps = psC.tile([1, n_out], fp32, tag="fcps")
            for ch in range(n_ch):
                nc.tensor.matmul(
                    ps[:], v_cols[:, ch:ch + 1], w_sb[:, ch, :],
                    start=(ch == 0), stop=(ch == n_ch - 1),
                )
            b_sb = w1.tile([1, n_out], fp32, tag=f"{tag}_b")
            nc.sync.dma_start(b_sb[:], b_d[:, :])
            r = w1.tile([1, n_out], fp32, tag=f"{tag}_r")
            nc.vector.tensor_tensor(r[:], ps[:], b_sb[:], op=OP.add)
            if relu:
                nc.vector.tensor_scalar_max(r[:], r[:], 0.0)
            return r

        def row_to_cols(v_row, n_ch, tag):
            """[1, 128*n_ch] -> [128, n_ch] via PE transposes."""
            cols = w1.tile([128, n_ch], fp32, tag=tag)
            for j in range(n_ch):
                tpv = psC.tile([128, 1], fp32, tag="tpv")
                nc.tensor.transpose(
                    tpv[:], v_row[:, ts(j, 128)], ident_sb[0:1, 0:1]
                )
                nc.vector.tensor_copy(cols[:, j:j + 1], tpv[:])
            return cols

        v1 = fc_row(u_t, 9, L1T, 512, b8r, True, "fc1")
        v1c = row_to_cols(v1, 4, "v1c")
        v2 = fc_row(v1c, 4, L2T, 256, b9r, True, "fc2")
        v2c = row_to_cols(v2, 2, "v2c")
        v3 = fc_row(v2c, 2, L3T, 28, bL3r, False, "fc3")
        nc.sync.dma_start(out_t[:, :], v3[:])

        for p in (psC, psA, w1, dramp, xcmp, consts):
            p.release()

    nc.compile()
    return nc


_PROGRAM_CACHE = {}


def get_program(n_cores=NCORES, debug=False):
    key = (n_cores, debug)
    if key not in _PROGRAM_CACHE:
        _PROGRAM_CACHE[key] = _build_program(n_cores, debug)
    return _PROGRAM_CACHE[key]


def make_in_maps(inputs, n_cores=NCORES):
    """Host-side preprocessing: fold BN into weights, build per-core inputs."""
    f32 = np.float32

    def arr(v):
        return np.ascontiguousarray(np.asarray(v), dtype=f32)

    x = arr(inputs["x"])  # [B, 3, N]
    lmat = arr(inputs["l"])  # [B, 5]
    nmat = arr(inputs["n"])  # [B, 7]

    def fold(g):
        return arr(g) / np.sqrt(f32(1.0) + f32(EPS), dtype=f32)

    common = {}
    for li, (wn, gn, bn) in enumerate(
        [("W1", "g1", "b1"), ("W2", "g2", "b2"), ("W3", "g3", "b3"),
         ("W4", "g4", "b4")]
    ):
        W = arr(inputs[wn])  # [O, 2C]
        s = fold(inputs[gn])
        b = arr(inputs[bn])
        C = W.shape[1] // 2
        Wn = W[:, :C] * s[:, None]
        Wc = W[:, C:] * s[:, None]
        common[f"wy{li}"] = arr(Wn.T)
        common[f"wz{li}"] = arr(
            np.concatenate([(Wc - Wn).T, np.zeros((1, len(b)), f32), b[None, :]],
                           axis=0))

    s5 = fold(inputs["g5"])
    common["w5"] = arr((arr(inputs["W5"]) * s5[:, None]).T)
    common["b5r"] = arr(inputs["b5"])[None, :]
    s6 = fold(inputs["g6"])
    common["w6T"] = arr((arr(inputs["W6"]) * s6[:, None]).T)
    common["b6c"] = arr(inputs["b6"])[:, None]
    s7 = fold(inputs["g7"])
    common["w7T"] = arr((arr(inputs["W7"]) * s7[:, None]).T)
    common["b7c"] = arr(inputs["b7"])[:, None]
    s8 = fold(inputs["g8"])
    common["L1T"] = arr((arr(inputs["L1"]) * s8[:, None]).T)
    common["b8r"] = arr(inputs["b8"])[None, :]
    s9 = fold(inputs["g9"])
    common["L2T"] = arr((arr(inputs["L2"]) * s9[:, None]).T)
    common["b9r"] = arr(s9 * arr(inputs["L2b"]) + arr(inputs["b9"]))[None, :]
    common["L3T"] = arr(arr(inputs["L3"]).T)
    common["bL3r"] = arr(inputs["L3b"])[None, :]
    common["ident"] = np.eye(128, dtype=f32)

    in_maps = []
    for core in range(n_cores):
        b_i = (core // 2) % B
        m = dict(common)
        m["x0"] = arr(x[b_i])
        m["lvec"] = arr(lmat[b_i])[:, None]
        m["nvec"] = arr(nmat[b_i])[:, None]
        in_maps.append(m)
    return in_maps


LAST_RESULTS = None


def kernel(**inputs):
    global LAST_RESULTS
    from concourse.bass_utils import run_bass_kernel_spmd

    nc = get_program(NCORES)
    in_maps = make_in_maps(inputs, NCORES)
    res = run_bass_kernel_spmd(nc, in_maps, core_ids=list(range(NCORES)))
    LAST_RESULTS = res
    rows = [res.results[2 * b]["out"].reshape(28) for b in range(B)]
    return np.stack(rows, axis=0).astype(np.float32)

